# revision 17
# baseline (speedup 1.0000x reference)
"""GCNConv-with-constraint kernel for 8 Trainium2 NeuronCores.

Strategy: nodes are sharded across the 8 cores by destination. The whole
(fp16) x table fits in SBUF (50048 x 128 x 2B = 12.8MB of the 24MB SBUF), so
per-edge source rows are fetched with one-hot PE matmuls against on-chip
128-row table blocks instead of descriptor-per-row SWDGE dma_gather (which is
latency-bound at ~1us/row on this platform and dominated the old kernel).

Per core:
  - dsts are split into superblocks of 512 (one PSUM bank of fp32 [128ch,512]).
  - edges of a superblock are bucketed by source block (128 table rows),
    buckets padded to a multiple of 64 so every 64-slot chunk is block-pure
    and the gather matmul writes a 64-aligned PSUM partition range (PE
    base-partition constraint: offsets 0/64 only).
  - per 128-edge tile:
      DVE  builds F_T[e, r] = (r == srcloc_e) * norm_e   (one-hot, norm-scaled)
           and sel[e, d] = (d == dstloc_e) via per-tile tensor_scalar ops
           (per-partition f32 scalar port; broadcast APs would drop DVE to
           1 elem/cycle by disabling the 2x/4x modes)
      PE   transposes F_T -> F[r, e] (identity matmul),
           2 gather matmuls  msgs[64-chunk, c] = F[:, chunk]^T @ xblk[s]
           1 scatter matmul  agg[c, d] += msgs^T @ sel   (PSUM accumulate)
      Act  copies F and msgs PSUM->SBUF (fp16)
  - superblock epilogue: agg -> SBUF, out^T = WnT^T @ agg + b, DMA out.

The x table is per-core ROTATED by the core's node offset so self-loop edges
(dst d -> table row d_local) hit core-independent buckets; bucket sizes are
maxed over the 8 cores so the single SPMD program is valid for all cores.
Host does structure/metadata only: degree bincount, norm coefficients, edge
sort/padding, W column-renorm (128x128), fp16 casts, final transpose/concat.
"""

import math
import os
from contextlib import ExitStack, nullcontext

import numpy as np

import concourse.bass as bass
import concourse.tile as tile
from concourse import bacc, mybir
from concourse.bass_utils import run_bass_kernel_spmd

N_CORES = 8
C = 128  # in/out channels
P = 128  # partitions / edge-tile size
SBW = 512  # dst superblock width (one fp32 PSUM bank)
CHUNK = 64  # gather sub-matmul width (PSUM base partition must be 0/32/64)
QUAD = 4  # tiles processed per group (shared DVE ops / Act copies)

f16 = mybir.dt.float16
f32 = mybir.dt.float32

# test.py introspection: the last BassKernelResults
LAST_RESULTS = None


def _prep(x, edge_index, W, b):
    """Host-side sharding/metadata prep. Returns per-core input maps and the
    common (data-dependent, core-uniform) structure baked into the program."""
    x = np.asarray(x)
    N = x.shape[0]
    assert N % N_CORES == 0, N
    npc = N // N_CORES
    NSB = math.ceil(npc / SBW)
    NBLK = math.ceil(N / P)
    Npad = NBLK * P
    nk = NSB * NBLK

    src = np.asarray(edge_index[0], dtype=np.int64)
    dst = np.asarray(edge_index[1], dtype=np.int64)

    deg = np.bincount(dst, minlength=N).astype(np.float64) + 1.0
    dinv = 1.0 / np.sqrt(deg)
    norm = (dinv[src] * dinv[dst]).astype(np.float32)

    core = dst // npc
    dstl = dst - core * npc
    r = (src - core * npc) % N  # rotated table row of the source

    key = (dstl >> 9) * NBLK + (r >> 7)

    # self loops: dst d -> rotated row d_local; identical structure on all cores
    arN = np.arange(npc, dtype=np.int64)
    self_key = (arN >> 9) * NBLK + (arN >> 7)
    cnt_self = np.bincount(self_key, minlength=nk)

    cnt = np.zeros((N_CORES, nk), dtype=np.int64)
    for c in range(N_CORES):
        cnt[c] = np.bincount(key[core == c], minlength=nk) + cnt_self
    m = ((cnt.max(axis=0) + CHUNK - 1) // CHUNK) * CHUNK  # padded bucket sizes

    # per-SB chunk lists (+pad chunks so each SB is a whole number of QUADs)
    tiles_sb = []
    tile_off = []
    chunk_block = []  # per SB: list of source-block ids, one per CHUNK-slot chunk
    bucket_off = np.zeros(nk, dtype=np.int64)  # global slot offset per bucket
    toff = 0
    for sb in range(NSB):
        msb = m[sb * NBLK : (sb + 1) * NBLK]
        offs = np.concatenate([[0], np.cumsum(msb)[:-1]])
        bucket_off[sb * NBLK : (sb + 1) * NBLK] = toff * P + offs
        slots = int(msb.sum())
        slots_pad = math.ceil(slots / (P * QUAD)) * (P * QUAD)
        blocks = []
        for s in range(NBLK):
            blocks.extend([s] * (int(msb[s]) // CHUNK))
        blocks.extend([0] * ((slots_pad - slots) // CHUNK))
        chunk_block.append(blocks)
        tile_off.append(toff)
        tiles_sb.append(slots_pad // P)
        toff += slots_pad // P
    n_tiles = toff
    n_slots = n_tiles * P

    # host W renorm: Wn = W * min(1, 1/||W[:,i]||); ship WnT = Wn^T [in, out]
    Wf = np.asarray(W, dtype=np.float64)
    norms = np.sqrt((Wf**2).sum(axis=0, keepdims=True))
    scale = np.where(norms > 1.0, 1.0 / norms, 1.0)
    WnT = np.ascontiguousarray((np.asarray(W, np.float32) * scale.astype(np.float32)).T)
    bvec = np.ascontiguousarray(np.asarray(b, dtype=np.float32).reshape(C, 1))

    iotaP = np.ascontiguousarray(
        np.broadcast_to(np.arange(P, dtype=np.float16)[None, :], (P, P))
    )
    iotaS = np.ascontiguousarray(
        np.broadcast_to(np.arange(SBW, dtype=np.float16)[None, :], (P, SBW))
    )
    ident16 = np.eye(P, dtype=np.float16)

    in_maps = []
    for c in range(N_CORES):
        mask = core == c
        lo = c * npc
        allr = np.concatenate([r[mask], arN])
        alldl = np.concatenate([dstl[mask], arN])
        allnm = np.concatenate([norm[mask], (dinv[lo : lo + npc] ** 2).astype(np.float32)])
        allkey = np.concatenate([key[mask], self_key])

        order = np.argsort(allkey, kind="stable")
        cntc = np.bincount(allkey, minlength=nk)
        starts = np.concatenate([[0], np.cumsum(cntc)[:-1]])
        rank = np.arange(len(allkey)) - np.repeat(starts, cntc)
        slot = bucket_off[allkey[order]] + rank

        S = np.zeros(n_slots, np.float32)
        NM = np.zeros(n_slots, np.float32)
        D = np.full(n_slots, -1.0, np.float32)
        S[slot] = (allr[order] & (P - 1)).astype(np.float32)
        NM[slot] = allnm[order].astype(np.float32)
        D[slot] = (alldl[order] & (SBW - 1)).astype(np.float32)

        xrot = np.concatenate([x[lo:], x[:lo]]).astype(np.float16)
        if Npad > N:
            xrot = np.concatenate([xrot, np.zeros((Npad - N, C), np.float16)])
        xpm = np.ascontiguousarray(
            xrot.reshape(NBLK, P, C).transpose(1, 0, 2).reshape(P, NBLK * C)
        )

        in_maps.append(
            {
                "xtab": xpm,
                "srcloc": np.ascontiguousarray(S.reshape(n_tiles, P).T),
                "nrms": np.ascontiguousarray(NM.reshape(n_tiles, P).T),
                "dsts": np.ascontiguousarray(D.reshape(n_tiles, P).T),
                "wnT": WnT,
                "bvec": bvec,
                "iotaP": iotaP,
                "iotaS": iotaS,
                "ident16": ident16,
            }
        )

    structure = dict(
        N=N,
        npc=npc,
        NSB=NSB,
        NBLK=NBLK,
        n_tiles=n_tiles,
        tiles_sb=tiles_sb,
        tile_off=tile_off,
        chunk_block=chunk_block,
    )
    return in_maps, structure


def _build_program(st, repeat=1):
    N, NSB, NBLK, n_tiles = st["N"], st["NSB"], st["NBLK"], st["n_tiles"]
    nc = bacc.Bacc("TRN2", target_bir_lowering=False, debug=False, num_devices=N_CORES)

    xtab = nc.dram_tensor("xtab", [P, NBLK * C], f16, kind="ExternalInput").ap()
    srcloc = nc.dram_tensor("srcloc", [P, n_tiles], f32, kind="ExternalInput").ap()
    nrms = nc.dram_tensor("nrms", [P, n_tiles], f32, kind="ExternalInput").ap()
    dsts = nc.dram_tensor("dsts", [P, n_tiles], f32, kind="ExternalInput").ap()
    wnT = nc.dram_tensor("wnT", [C, C], f32, kind="ExternalInput").ap()
    bvec = nc.dram_tensor("bvec", [C, 1], f32, kind="ExternalInput").ap()
    iotaP = nc.dram_tensor("iotaP", [P, P], f16, kind="ExternalInput").ap()
    iotaS = nc.dram_tensor("iotaS", [P, SBW], f16, kind="ExternalInput").ap()
    ident16 = nc.dram_tensor("ident16", [P, P], f16, kind="ExternalInput").ap()
    outt = nc.dram_tensor("outt", [C, NSB * SBW], f32, kind="ExternalOutput").ap()

    Copy = mybir.ActivationFunctionType.Copy
    Op = mybir.AluOpType

    with tile.TileContext(nc) as tc, ExitStack() as ctx:
        cpool = ctx.enter_context(tc.tile_pool(name="const", bufs=1))
        xsb = cpool.tile([P, NBLK, C], f16, tag="xsb")
        nc.sync.dma_start(xsb[:], xtab[:])
        iotaP_sb = cpool.tile([P, P], f16, tag="iotaP")
        nc.sync.dma_start(iotaP_sb[:], iotaP[:])
        iotaS_sb = cpool.tile([P, SBW], f16, tag="iotaS")
        nc.sync.dma_start(iotaS_sb[:], iotaS[:])
        ident_sb = cpool.tile([P, P], f16, tag="ident16")
        nc.sync.dma_start(ident_sb[:], ident16[:])
        wnT_sb = cpool.tile([C, C], f32, tag="wnT")
        nc.sync.dma_start(wnT_sb[:], wnT[:])
        bias_sb = cpool.tile([C, 1], f32, tag="bias")
        nc.sync.dma_start(bias_sb[:], bvec[:])

        # all edge metadata is SBUF-resident (loaded once: ~15KB/partition)
        sl_all = cpool.tile([P, n_tiles], f32, tag="sl_all")
        nc.sync.dma_start(sl_all[:], srcloc[:])
        nm_all = cpool.tile([P, n_tiles], f32, tag="nm_all")
        nc.sync.dma_start(nm_all[:], nrms[:])
        dl_all = cpool.tile([P, n_tiles], f32, tag="dl_all")
        nc.sync.dma_start(dl_all[:], dsts[:])

        ftpool = ctx.enter_context(tc.tile_pool(name="ft", bufs=3))
        fsbpool = ctx.enter_context(tc.tile_pool(name="fsb", bufs=3))
        selpool = ctx.enter_context(tc.tile_pool(name="sel", bufs=3))
        msbpool = ctx.enter_context(tc.tile_pool(name="msb", bufs=3))
        asbpool = ctx.enter_context(tc.tile_pool(name="aggsb", bufs=2))
        fpsp = ctx.enter_context(tc.tile_pool(name="fps", bufs=2, space="PSUM"))
        mpsp = ctx.enter_context(tc.tile_pool(name="mps", bufs=2, space="PSUM"))
        aggp = ctx.enter_context(tc.tile_pool(name="aggps", bufs=2, space="PSUM"))
        outp = ctx.enter_context(tc.tile_pool(name="outps", bufs=1, space="PSUM"))

        out_stage = cpool.tile([C, NSB, SBW], f32, tag="out_stage")

        loop = tc.For_i(0, repeat) if repeat > 1 else nullcontext()
        with loop:
            for sb in range(NSB):
                t0 = st["tile_off"][sb]
                nt = st["tiles_sb"][sb]
                blocks = st["chunk_block"][sb]
                sl = sl_all[:, t0 : t0 + nt]
                nm = nm_all[:, t0 : t0 + nt]
                dl = dl_all[:, t0 : t0 + nt]

                agg = aggp.tile([C, SBW], f32, tag="agg", name=f"agg_sb{sb}")
                nq = nt // QUAD
                for q in range(nq):
                    ft = ftpool.tile([P, QUAD, P], f16, tag="ft")
                    for t in range(QUAD):
                        k = QUAD * q + t
                        nc.vector.tensor_scalar(
                            out=ft[:, t, :], in0=iotaP_sb[:],
                            scalar1=sl[:, k : k + 1], scalar2=nm[:, k : k + 1],
                            op0=Op.is_equal, op1=Op.mult)
                    fps = fpsp.tile([P, QUAD, P], f16, tag="fps")
                    for t in range(QUAD):
                        nc.tensor.transpose(fps[:, t, :], ft[:, t, :], ident_sb[:])
                    fsb = fsbpool.tile([P, QUAD, P], f16, tag="fsb")
                    nc.scalar.activation(fsb[:], fps[:], Copy)
                    sel = selpool.tile([P, QUAD, SBW], f16, tag="sel")
                    for t in range(QUAD):
                        k = QUAD * q + t
                        nc.vector.tensor_scalar(
                            out=sel[:, t, :], in0=iotaS_sb[:],
                            scalar1=dl[:, k : k + 1], scalar2=None,
                            op0=Op.is_equal)
                    mps = mpsp.tile([P, QUAD, C], f32, tag="mps")
                    for t in range(QUAD):
                        for g in range(P // CHUNK):
                            s = blocks[(QUAD * q + t) * (P // CHUNK) + g]
                            nc.tensor.matmul(
                                mps[CHUNK * g : CHUNK * (g + 1), t, :],
                                lhsT=fsb[:, t, CHUNK * g : CHUNK * (g + 1)],
                                rhs=xsb[:, s, :], start=True, stop=True)
                    msb = msbpool.tile([P, QUAD, C], f16, tag="msb")
                    nc.scalar.activation(msb[:], mps[:], Copy)
                    for t in range(QUAD):
                        nc.tensor.matmul(
                            agg[:], lhsT=msb[:, t, :], rhs=sel[:, t, :],
                            start=(q == 0 and t == 0),
                            stop=(q == nq - 1 and t == QUAD - 1))
                agg_sb = asbpool.tile([C, SBW], f32, tag="aggsb")
                nc.scalar.activation(agg_sb[:], agg[:], Copy)
                outT_ps = outp.tile([C, SBW], f32, tag="outps")
                nc.tensor.matmul(
                    outT_ps[:], lhsT=wnT_sb[:], rhs=agg_sb[:], start=True, stop=True
                )
                nc.vector.tensor_scalar(
                    out=out_stage[:, sb, :],
                    in0=outT_ps[:],
                    scalar1=bias_sb[:],
                    scalar2=None,
                    op0=Op.add,
                )
            # one fat output DMA per execution (128 descriptors of 26KB)
            nc.sync.dma_start(outt[:], out_stage[:])

    nc.compile()
    return nc


def kernel(x, edge_index, W, b):
    global LAST_RESULTS
    x = np.asarray(x)
    N = x.shape[0]
    assert x.shape[1] == C and W.shape == (C, C)

    in_maps, st = _prep(x, edge_index, W, b)
    nc = _build_program(st)

    os.environ.setdefault("BASS_NEVER_TRACE", "1")  # no NTFF hook in this env
    res = run_bass_kernel_spmd(nc, in_maps, list(range(N_CORES)))
    LAST_RESULTS = res

    npc = st["npc"]
    shards = []
    for s in range(N_CORES):
        lo = s * npc
        hi = min((s + 1) * npc, N)
        outT = res.results[s]["outt"]  # [C, NSB*SBW]
        shards.append(outT[:, : hi - lo].T)
    return np.ascontiguousarray(np.concatenate(shards, axis=0), dtype=np.float32)
